# revision 13
# baseline (speedup 1.0000x reference)
"""Hawk (RG-LRU) block kernel for Trainium2, SPMD over 8 NeuronCores.

Sharding: tokens. Core k handles batch b=k//2, half h=k%2 (2048 tokens).
Fused single pass per 512-token tile: xa-proj + gate-proj (f32r) share one
x stream; causal conv runs in bf16 on DVE; gates GEMM in bf16 (optionally
fp8 DoubleRow); activations batched per ACT table; diagonal recurrence via
hardware tensor_tensor_scan. h/p/gelu spill to DRAM in bf16; pass 2 applies
the cross-half carry correction (4KB pairwise AllReduce) and the f32r
output projection. Gates GEMM is software-pipelined one tile behind the
input projections to keep the PE array continuously fed.
"""
import sys

sys.path.insert(0, "/opt/trn_rl_repo")

import numpy as np
import ml_dtypes
from contextlib import ExitStack

import concourse.bass as bass
import concourse.tile as tile
import concourse.bacc as bacc
from concourse import mybir
from concourse.bass_utils import run_bass_kernel_spmd

F32 = mybir.dt.float32
F32R = mybir.dt.float32r
BF16 = mybir.dt.bfloat16
AF = mybir.ActivationFunctionType
OP = mybir.AluOpType

B, T, DIM = 4, 4096, 1024
E = 1024
KC = 4  # conv taps
N_CORES = 8
T_LOC = T // 2      # 2048 tokens per core
TT = 512            # token tile
NTT = T_LOC // TT   # 4
NE = E // 128       # 8 channel chunks
NK = DIM // 128     # 8 contraction tiles


def _build_kernel(profile_mode=False):
    nc = bacc.Bacc("TRN2", target_bir_lowering=False, debug=False,
                   num_devices=1 if profile_mode else N_CORES)

    xT = nc.dram_tensor("xT", [DIM, T_LOC], BF16, kind="ExternalInput")
    xa_halo = nc.dram_tensor("xa_halo", [E, KC - 1], BF16, kind="ExternalInput")
    w_in_cat = nc.dram_tensor("w_in_cat", [DIM, 2 * E], BF16, kind="ExternalInput")
    w_gates = nc.dram_tensor("w_gates", [E, 2 * E], BF16, kind="ExternalInput")
    w_out = nc.dram_tensor("w_out", [E, DIM], F32, kind="ExternalInput")
    wc = nc.dram_tensor("wc", [E, KC], F32, kind="ExternalInput")
    b_conv = nc.dram_tensor("b_conv", [E, 1], F32, kind="ExternalInput")
    neg_c = nc.dram_tensor("neg_c", [E, 1], F32, kind="ExternalInput")
    nch = nc.dram_tensor("nch", [E, 1], F32, kind="ExternalInput")
    mask_c = nc.dram_tensor("mask_c", [128, 1], F32, kind="ExternalInput")
    mask_u = nc.dram_tensor("mask_u", [128, 1], F32, kind="ExternalInput")
    out = nc.dram_tensor("out", [T_LOC, DIM], BF16, kind="ExternalOutput")

    with tile.TileContext(nc) as tc, ExitStack() as ctx:
        _body(ctx, tc, nc, profile_mode=profile_mode,
              xT=xT, xa_halo=xa_halo, w_in_cat=w_in_cat, w_gates=w_gates,
              w_out=w_out, wc=wc, b_conv=b_conv, neg_c=neg_c, nch=nch,
              mask_c=mask_c, mask_u=mask_u, out=out)
    nc.compile()
    return nc


def _body(ctx, tc, nc, *, xT, xa_halo, w_in_cat, w_gates, w_out, wc,
          b_conv, neg_c, nch, mask_c, mask_u, out, profile_mode=False):
    consts = ctx.enter_context(tc.tile_pool(name="consts", bufs=1))
    dram = ctx.enter_context(tc.tile_pool(name="dram", bufs=1, space="DRAM"))

    def load_chan_const(t_dram, n):
        t = consts.tile([128, NE, n], F32, tag=t_dram.name)
        nc.sync.dma_start(t[:], t_dram.ap().rearrange("(m p) n -> p m n", p=128))
        return t

    wc_sb = load_chan_const(wc, KC)
    b_conv_sb = load_chan_const(b_conv, 1)
    neg_c_sb = load_chan_const(neg_c, 1)
    nch_sb = load_chan_const(nch, 1)
    mc_sb = consts.tile([128, 1], F32, tag="mc")
    nc.sync.dma_start(mc_sb[:], mask_c.ap()[:])
    mu_sb = consts.tile([128, 1], F32, tag="mu")
    nc.sync.dma_start(mu_sb[:], mask_u.ap()[:])
    zeros = consts.tile([128, TT], F32, tag="zeros")
    nc.vector.memset(zeros[:], 0.0)
    c_zero = consts.tile([128, 1], F32, tag="c_zero")
    nc.vector.memset(c_zero[:], 0.0)
    c_sqb = consts.tile([128, 1], F32, tag="c_sqb")
    nc.vector.memset(c_sqb[:], 1.000001)
    hl = consts.tile([128, NE], F32, tag="hl")
    pl = consts.tile([128, NE], F32, tag="pl")
    carry = consts.tile([128, NE], F32, tag="carry")

    h_dram = dram.tile([NTT, 128, NE, TT], BF16, tag="h_spill")
    p_dram = dram.tile([NTT, 128, NE, TT], BF16, tag="p_spill")
    g_dram = dram.tile([NTT, 128, NE, TT], BF16, tag="g_spill")
    cc_in = dram.tile([E], F32, tag="cc_in")
    cc_out = dram.tile([E], F32, tag="cc_out")

    hout = ctx.enter_context(tc.tile_pool(name="hout", bufs=1))
    pout = ctx.enter_context(tc.tile_pool(name="pout", bufs=1))
    gout = ctx.enter_context(tc.tile_pool(name="gout", bufs=1))

    # ---- weights (resident through pass 1) ----
    p1 = ExitStack()
    win_pool = p1.enter_context(tc.tile_pool(name="w_in", bufs=1, side="right"))
    win_sb = win_pool.tile([128, NK, 2 * E], BF16)
    win_src = w_in_cat.ap().rearrange("(k p) f -> p k f", p=128)
    wg_pool = p1.enter_context(tc.tile_pool(name="w_gates", bufs=1, side="right"))
    wg_sb = wg_pool.tile([128, NK, 2 * E], BF16)
    wg_src = w_gates.ap().rearrange("(k p) f -> p k f", p=128)

    xpool = p1.enter_context(tc.tile_pool(name="xs", bufs=2))
    xa_pool = p1.enter_context(tc.tile_pool(name="xa", bufs=2))
    xc_pool = p1.enter_context(tc.tile_pool(name="xc", bufs=2))
    sig_pool = p1.enter_context(tc.tile_pool(name="sig", bufs=8))
    apool = p1.enter_context(tc.tile_pool(name="alpha", bufs=8))
    wpool = p1.enter_context(tc.tile_pool(name="work", bufs=2))
    ps1 = p1.enter_context(tc.tile_pool(name="ps1", bufs=8, space="PSUM"))

    h_t = hout.tile([128, NE, TT], BF16)
    p_t = pout.tile([128, NE, TT], BF16)
    gel3 = gout.tile([128, NE, TT], BF16, tag="gel3", bufs=1)

    xa_tiles = []   # per-tt xa_ext handles for tail chaining
    xc_tiles = {}   # tt -> xcb tile

    def emit_proj(tt):
        """x load + xa/gate projections + gelu + conv for tile tt."""
        xt = xpool.tile([128, NK, TT], BF16, tag="xt")
        xsrc = xT.ap().rearrange("(k p) t -> p k t", p=128)
        if tt == 0:
            for k in range(NK):
                nc.sync.dma_start(win_sb[:, k], win_src[:, k])
            for k in range(NK):
                nc.sync.dma_start(xt[:, k], xsrc[:, k, 0:TT])
        else:
            nc.sync.dma_start(xt[:], xsrc[:, :, tt * TT:(tt + 1) * TT])
        xa_ext = xa_pool.tile([128, NE, TT + KC - 1], BF16, tag="xa")
        xa_tiles.append(xa_ext)
        if tt == 0:
            nc.sync.dma_start(
                xa_ext[:, :, 0:KC - 1],
                xa_halo.ap().rearrange("(m p) n -> p m n", p=128))
        else:
            nc.vector.tensor_copy(xa_ext[:, :, 0:KC - 1],
                                  xa_tiles[tt - 1][:, :, TT:TT + KC - 1])
        xcb = xc_pool.tile([128, NE, TT], BF16, tag="xcb")
        xc_tiles[tt] = xcb
        for m in range(NE):
            pxa = ps1.tile([128, TT], F32, tag="ps")
            for k in range(NK):
                nc.tensor.matmul(pxa[:], win_sb[:, k, E + m * 128:E + (m + 1) * 128],
                                 xt[:, k], start=(k == 0), stop=(k == NK - 1))
            pg = ps1.tile([128, TT], F32, tag="ps")
            for k in range(NK):
                nc.tensor.matmul(pg[:], win_sb[:, k, m * 128:(m + 1) * 128],
                                 xt[:, k], start=(k == 0), stop=(k == NK - 1))
            nc.scalar.copy(xa_ext[:, m, KC - 1:TT + KC - 1], pxa[:])
            if tt == NTT - 1:
                nc.scalar.activation(gel3[:, m], pg[:], AF.Gelu, bias=c_zero[:])
            else:
                gel = gout.tile([128, TT], BF16, tag="gel", bufs=3)
                nc.scalar.activation(gel[:], pg[:], AF.Gelu, bias=c_zero[:])
                nc.sync.dma_start(g_dram[tt, :, m], gel[:])
            # causal depthwise conv, 4 taps, bf16 accumulate
            nc.vector.tensor_scalar(
                xcb[:, m], xa_ext[:, m, 0:TT], wc_sb[:, m, 0:1],
                b_conv_sb[:, m, 0:1], op0=OP.mult, op1=OP.add)
            for j in range(1, KC):
                nc.vector.scalar_tensor_tensor(
                    xcb[:, m], xa_ext[:, m, j:j + TT], wc_sb[:, m, j:j + 1],
                    xcb[:, m], op0=OP.mult, op1=OP.add)
        if tt == 0:
            for k in range(NK):
                nc.sync.dma_start(wg_sb[:, k], wg_src[:, k])

    def emit_gates(tt):
        """gates GEMM + activations + scans + spills for tile tt."""
        xcb = xc_tiles.pop(tt)
        pfs, pis = [], []
        for m in range(NE):
            pf = ps1.tile([128, TT], F32, tag="ps")
            for k in range(NK):
                nc.tensor.matmul(pf[:], wg_sb[:, k, m * 128:(m + 1) * 128],
                                 xcb[:, k], start=(k == 0), stop=(k == NK - 1))
            pfs.append(pf)
            pi = ps1.tile([128, TT], F32, tag="ps")
            for k in range(NK):
                nc.tensor.matmul(pi[:], wg_sb[:, k, E + m * 128:E + (m + 1) * 128],
                                 xcb[:, k], start=(k == 0), stop=(k == NK - 1))
            pis.append(pi)
        # sigmoid(x) = 0.5*tanh(x/2) + 0.5: Tanh and Exp share an ACT table,
        # so alpha = exp(c*sig(f)) = exp(c/2*tanh(f/2) + c/2) and
        # alpha^2 = exp(c*tanh(f/2) + c) cost zero table switches; only the
        # final Sqrt switches tables.
        sis, alphas, betas = {}, {}, {}
        for m in range(NE):
            thf = wpool.tile([128, TT], F32, tag="thf", bufs=2)
            nc.scalar.activation(thf[:], pfs[m][:], AF.Tanh,
                                 scale=0.5, bias=c_zero[:])
            thi = wpool.tile([128, TT], F32, tag="thi", bufs=2)
            nc.scalar.activation(thi[:], pis[m][:], AF.Tanh,
                                 scale=0.5, bias=c_zero[:])
            alpha = apool.tile([128, TT], F32, tag="alpha")
            nc.scalar.activation(alpha[:], thf[:], AF.Exp,
                                 scale=nch_sb[:, m, 0:1],
                                 bias=nch_sb[:, m, 0:1])
            alphas[m] = alpha
            asq = wpool.tile([128, TT], F32, tag="asq", bufs=8)
            nc.scalar.activation(asq[:], thf[:], AF.Exp,
                                 scale=neg_c_sb[:, m, 0:1],
                                 bias=neg_c_sb[:, m, 0:1])
            betas[m] = asq
            si = sig_pool.tile([128, TT], F32, tag="si")
            nc.vector.tensor_scalar(si[:], thi[:], 0.5, 0.5,
                                    op0=OP.mult, op1=OP.add)
            sis[m] = si
        # sq_scale == -1.0, but computed from the last Exp output so every
        # Sqrt becomes ready only after the whole Exp phase — stops the
        # scheduler from interleaving Sqrt/Exp table reloads.
        sq_scale = wpool.tile([128, 1], F32, tag="sqs", bufs=2)
        nc.vector.tensor_scalar(sq_scale[:], betas[NE - 1][:, 0:1], 0.0, -1.0,
                                op0=OP.mult, op1=OP.add)
        for m in range(NE):
            nc.scalar.activation(betas[m][:], betas[m][:], AF.Sqrt,
                                 bias=c_sqb[:], scale=sq_scale[:, 0:1])
        for m in range(NE):
            bsi = wpool.tile([128, TT], F32, tag="bsi", bufs=2)
            nc.gpsimd.tensor_mul(bsi[:], betas[m][:], sis[m][:])
            u = wpool.tile([128, TT], F32, tag="u", bufs=2)
            nc.vector.tensor_mul(u[:], bsi[:], xcb[:, m])
            nc.vector.tensor_tensor_scan(
                h_t[:, m], alphas[m][:], u[:],
                0.0 if tt == 0 else hl[:, m:m + 1],
                op0=OP.mult, op1=OP.add)
            nc.vector.tensor_copy(hl[:, m:m + 1], h_t[:, m, TT - 1:TT])
            nc.vector.tensor_tensor_scan(
                p_t[:, m], alphas[m][:], zeros[:],
                1.0 if tt == 0 else pl[:, m:m + 1],
                op0=OP.mult, op1=OP.add)
            nc.vector.tensor_copy(pl[:, m:m + 1], p_t[:, m, TT - 1:TT])
        if tt != NTT - 1:
            nc.sync.dma_start(h_dram[tt], h_t[:])
            nc.sync.dma_start(p_dram[tt], p_t[:])

    # pass 1, gates pipelined one tile behind the projections
    emit_proj(0)
    for tt in range(1, NTT):
        emit_proj(tt)
        emit_gates(tt - 1)
    emit_gates(NTT - 1)

    # ---- carry exchange (4KB pairwise AllReduce) ----
    contrib = consts.tile([128, NE], F32, tag="contrib")
    nc.vector.tensor_scalar(contrib[:], hl[:], mc_sb[:, 0:1], None, op0=OP.mult)
    nc.sync.dma_start(cc_in[:].rearrange("(j p) -> p j", p=128), contrib[:])
    if profile_mode:
        nc.sync.dma_start(cc_out[:], cc_in[:])
    else:
        nc.gpsimd.collective_compute(
            "AllReduce", OP.add,
            replica_groups=[[0, 1], [2, 3], [4, 5], [6, 7]],
            ins=[cc_in[:].opt()], outs=[cc_out[:].opt()])
    craw = consts.tile([128, NE], F32, tag="craw")
    nc.sync.dma_start(craw[:], cc_out[:].rearrange("(j p) -> p j", p=128))
    nc.vector.tensor_scalar(carry[:], craw[:], mu_sb[:, 0:1], None, op0=OP.mult)

    p1.close()

    # ---- pass 2: carry correction + y + output projection ----
    with ExitStack() as p2:
        wo_pool = p2.enter_context(tc.tile_pool(name="w_out", bufs=1, side="right"))
        wo_sb = wo_pool.tile([128, NE, DIM], F32R)
        wo_src = w_out.ap().rearrange("(m p) c -> p m c", p=128)
        for m in range(NE):
            nc.sync.dma_start(wo_sb[:, m], wo_src[:, m].bitcast(F32R))
        gin_pool = p2.enter_context(tc.tile_pool(name="gin", bufs=2))
        hin_pool = p2.enter_context(tc.tile_pool(name="hin", bufs=2))
        pin_pool = p2.enter_context(tc.tile_pool(name="pin", bufs=2))
        ht_pool = p2.enter_context(tc.tile_pool(name="htp", bufs=3))
        y_pool = p2.enter_context(tc.tile_pool(name="y", bufs=9))
        osb_pool = p2.enter_context(tc.tile_pool(name="osb", bufs=3))
        ps2 = p2.enter_context(tc.tile_pool(name="ps2", bufs=8, space="PSUM"))
        for tt in [NTT - 1] + list(range(NTT - 1)):
            if tt == NTT - 1:
                gin, hin, pin = gel3, h_t, p_t
            else:
                gin = gin_pool.tile([128, NE, TT], BF16, tag="gin")
                nc.sync.dma_start(gin[:], g_dram[tt])
                hin = hin_pool.tile([128, NE, TT], BF16, tag="hin")
                nc.sync.dma_start(hin[:], h_dram[tt])
                pin = pin_pool.tile([128, NE, TT], BF16, tag="pin")
                nc.sync.dma_start(pin[:], p_dram[tt])
            ys = []
            for m in range(NE):
                htrue = ht_pool.tile([128, TT], BF16, tag="htrue")
                nc.vector.scalar_tensor_tensor(
                    htrue[:], pin[:, m], carry[:, m:m + 1], hin[:, m],
                    op0=OP.mult, op1=OP.add)
                y = y_pool.tile([128, TT], F32R, tag="y")
                nc.vector.tensor_mul(y[:], gin[:, m], htrue[:])
                ys.append(y)
            for q in range(TT // 128):
                pos = [ps2.tile([128, 512], F32, tag="ps", name=f"po{n}") for n in range(2)]
                for m in range(NE):
                    for n in range(2):
                        nc.tensor.matmul(
                            pos[n][:], ys[m][:, q * 128:(q + 1) * 128],
                            wo_sb[:, m, n * 512:(n + 1) * 512],
                            start=(m == 0), stop=(m == NE - 1))
                osb = osb_pool.tile([128, DIM], BF16, tag="osb")
                for n in range(2):
                    nc.scalar.copy(osb[:, n * 512:(n + 1) * 512], pos[n][:])
                nc.sync.dma_start(
                    out.ap()[tt * TT + q * 128:tt * TT + (q + 1) * 128, :],
                    osb[:])


_NC_CACHE = {}


def _get_nc():
    if "nc" not in _NC_CACHE:
        _NC_CACHE["nc"] = _build_kernel()
    return _NC_CACHE["nc"]


def _softplus(x):
    return np.logaddexp(0.0, x)


def kernel(x, w_in, w_conv, b_conv, w_gates, b_gates, forget_base, w_out,
           _want_trace=False):
    x = np.asarray(x, dtype=np.float32)
    w_in = np.asarray(w_in, dtype=np.float32)
    w_conv = np.asarray(w_conv, dtype=np.float32)
    b_conv = np.asarray(b_conv, dtype=np.float32)
    w_gates = np.asarray(w_gates, dtype=np.float32)
    forget_base = np.asarray(forget_base, dtype=np.float32)
    w_out = np.asarray(w_out, dtype=np.float32)

    nc = _get_nc()

    # [DIM, 2E]: cols 0:E gate-branch, E:2E x-branch
    w_in_cat = np.ascontiguousarray(
        np.concatenate([w_in[:E].T, w_in[E:].T], axis=1)).astype(
            ml_dtypes.bfloat16)
    w_gates_T = np.ascontiguousarray(w_gates.T).astype(ml_dtypes.bfloat16)
    w_out_T = np.ascontiguousarray(w_out.T)            # [E, DIM]
    wc_r = np.ascontiguousarray(w_conv.reshape(E, KC))
    neg_c = (-8.0 * _softplus(forget_base.astype(np.float64))).astype(
        np.float32)[:, None]

    common = {
        "w_in_cat": w_in_cat, "w_gates": w_gates_T, "w_out": w_out_T,
        "wc": wc_r, "b_conv": b_conv[:, None].copy(), "neg_c": neg_c,
        "nch": (0.5 * neg_c).copy(),
    }
    in_maps = []
    for k in range(N_CORES):
        b, half = k // 2, k % 2
        t0 = half * T_LOC
        xT_loc = np.ascontiguousarray(x[b, t0:t0 + T_LOC, :].T).astype(
            ml_dtypes.bfloat16)
        if half == 1:
            # xa for the 3 tokens before this chunk (for the causal conv)
            xa_halo = (x[b, t0 - (KC - 1):t0, :] @ w_in[E:].T).T
            xa_halo = np.ascontiguousarray(xa_halo).astype(ml_dtypes.bfloat16)
        else:
            xa_halo = np.zeros((E, KC - 1), dtype=ml_dtypes.bfloat16)
        mc = np.full((128, 1), 1.0 if half == 0 else 0.0, dtype=np.float32)
        mu = np.full((128, 1), 0.0 if half == 0 else 1.0, dtype=np.float32)
        in_maps.append({**common, "xT": xT_loc, "xa_halo": xa_halo,
                        "mask_c": mc, "mask_u": mu})

    res = run_bass_kernel_spmd(nc, in_maps, core_ids=list(range(N_CORES)),
                               trace=_want_trace)
    out_full = np.empty((B, T, DIM), dtype=np.float32)
    for k in range(N_CORES):
        b, half = k // 2, k % 2
        out_full[b, half * T_LOC:(half + 1) * T_LOC, :] = \
            res.results[k]["out"].astype(np.float32)
    if _want_trace:
        return out_full, res
    return out_full


# revision 18
# speedup vs baseline: 1.0163x; 1.0163x over previous
"""Hawk (RG-LRU) block kernel for Trainium2, SPMD over 8 NeuronCores.

Sharding: tokens. Core k handles batch b=k//2, half h=k%2 (2048 tokens).
Fused single pass per 512-token tile: xa-proj + gate-proj (f32r) share one
x stream; causal conv runs in bf16 on DVE; gates GEMM in bf16 (optionally
fp8 DoubleRow); activations batched per ACT table; diagonal recurrence via
hardware tensor_tensor_scan. h/p/gelu spill to DRAM in bf16; pass 2 applies
the cross-half carry correction (4KB pairwise AllReduce) and the f32r
output projection. Gates GEMM is software-pipelined one tile behind the
input projections to keep the PE array continuously fed.
"""
import sys

sys.path.insert(0, "/opt/trn_rl_repo")

import numpy as np
import ml_dtypes
from contextlib import ExitStack

import concourse.bass as bass
import concourse.tile as tile
import concourse.bacc as bacc
from concourse import mybir
from concourse.bass_utils import run_bass_kernel_spmd

F32 = mybir.dt.float32
F32R = mybir.dt.float32r
FP8 = mybir.dt.float8e4
BF16 = mybir.dt.bfloat16
AF = mybir.ActivationFunctionType
OP = mybir.AluOpType

B, T, DIM = 4, 4096, 1024
E = 1024
KC = 4  # conv taps
N_CORES = 8
T_LOC = T // 2      # 2048 tokens per core
TT = 512            # token tile
NTT = T_LOC // TT   # 4
NE = E // 128       # 8 channel chunks
NK = DIM // 128     # 8 contraction tiles


def _build_kernel(profile_mode=False):
    nc = bacc.Bacc("TRN2", target_bir_lowering=False, debug=False,
                   num_devices=1 if profile_mode else N_CORES)

    xT = nc.dram_tensor("xT", [DIM, T_LOC], BF16, kind="ExternalInput")
    xa_halo = nc.dram_tensor("xa_halo", [E, KC - 1], BF16, kind="ExternalInput")
    w_in_cat = nc.dram_tensor("w_in_cat", [DIM, 2 * E], BF16, kind="ExternalInput")
    w_gates = nc.dram_tensor("w_gates", [E, 2 * E], BF16, kind="ExternalInput")
    w_out = nc.dram_tensor("w_out", [E, DIM], BF16, kind="ExternalInput")
    wc = nc.dram_tensor("wc", [E, KC], F32, kind="ExternalInput")
    b_conv = nc.dram_tensor("b_conv", [E, 1], F32, kind="ExternalInput")
    neg_c = nc.dram_tensor("neg_c", [E, 1], F32, kind="ExternalInput")
    nch = nc.dram_tensor("nch", [E, 1], F32, kind="ExternalInput")
    mask_c = nc.dram_tensor("mask_c", [128, 1], F32, kind="ExternalInput")
    mask_u = nc.dram_tensor("mask_u", [128, 1], F32, kind="ExternalInput")
    out = nc.dram_tensor("out", [T_LOC, DIM], BF16, kind="ExternalOutput")

    with tile.TileContext(nc) as tc, ExitStack() as ctx:
        _body(ctx, tc, nc, profile_mode=profile_mode,
              xT=xT, xa_halo=xa_halo, w_in_cat=w_in_cat, w_gates=w_gates,
              w_out=w_out, wc=wc, b_conv=b_conv, neg_c=neg_c, nch=nch,
              mask_c=mask_c, mask_u=mask_u, out=out)
    nc.compile()
    return nc


def _body(ctx, tc, nc, *, xT, xa_halo, w_in_cat, w_gates, w_out, wc,
          b_conv, neg_c, nch, mask_c, mask_u, out, profile_mode=False):
    consts = ctx.enter_context(tc.tile_pool(name="consts", bufs=1))
    dram = ctx.enter_context(tc.tile_pool(name="dram", bufs=1, space="DRAM"))

    def load_chan_const(t_dram, n):
        t = consts.tile([128, NE, n], F32, tag=t_dram.name)
        nc.sync.dma_start(t[:], t_dram.ap().rearrange("(m p) n -> p m n", p=128))
        return t

    wc_sb = load_chan_const(wc, KC)
    b_conv_sb = load_chan_const(b_conv, 1)
    neg_c_sb = load_chan_const(neg_c, 1)
    nch_sb = load_chan_const(nch, 1)
    mc_sb = consts.tile([128, 1], F32, tag="mc")
    nc.sync.dma_start(mc_sb[:], mask_c.ap()[:])
    mu_sb = consts.tile([128, 1], F32, tag="mu")
    nc.sync.dma_start(mu_sb[:], mask_u.ap()[:])
    zeros = consts.tile([128, TT], F32, tag="zeros")
    nc.vector.memset(zeros[:], 0.0)
    c_zero = consts.tile([128, 1], F32, tag="c_zero")
    nc.vector.memset(c_zero[:], 0.0)
    c_sqb = consts.tile([128, 1], F32, tag="c_sqb")
    nc.vector.memset(c_sqb[:], 1.000001)
    hl = consts.tile([128, NE], F32, tag="hl")
    pl = consts.tile([128, NE], F32, tag="pl")
    carry = consts.tile([128, NE], F32, tag="carry")

    h_dram = dram.tile([NTT, 128, NE, TT], BF16, tag="h_spill")
    p_dram = dram.tile([NTT, 128, NE, TT], BF16, tag="p_spill")
    g_dram = dram.tile([NTT, 128, NE, TT], BF16, tag="g_spill")
    cc_in = dram.tile([E], F32, tag="cc_in")
    cc_out = dram.tile([E], F32, tag="cc_out")

    hout = ctx.enter_context(tc.tile_pool(name="hout", bufs=1))
    pout = ctx.enter_context(tc.tile_pool(name="pout", bufs=1))
    gout = ctx.enter_context(tc.tile_pool(name="gout", bufs=1))

    # ---- weights (resident through pass 1) ----
    p1 = ExitStack()
    win_pool = p1.enter_context(tc.tile_pool(name="w_in", bufs=1, side="right"))
    win_sb = win_pool.tile([128, NK, 2 * E], BF16)
    win_src = w_in_cat.ap().rearrange("(k p) f -> p k f", p=128)
    wg_pool = p1.enter_context(tc.tile_pool(name="w_gates", bufs=1, side="right"))
    wg_sb = wg_pool.tile([128, NK, 2 * E], BF16)
    wg_src = w_gates.ap().rearrange("(k p) f -> p k f", p=128)

    xpool = p1.enter_context(tc.tile_pool(name="xs", bufs=2))
    xa_pool = p1.enter_context(tc.tile_pool(name="xa", bufs=2))
    xc_pool = p1.enter_context(tc.tile_pool(name="xc", bufs=2))
    sig_pool = p1.enter_context(tc.tile_pool(name="sig", bufs=8))
    apool = p1.enter_context(tc.tile_pool(name="alpha", bufs=8))
    wpool = p1.enter_context(tc.tile_pool(name="work", bufs=2))
    ps1 = p1.enter_context(tc.tile_pool(name="ps1", bufs=8, space="PSUM"))

    h_t = hout.tile([128, NE, TT], BF16)
    p_t = pout.tile([128, NE, TT], BF16)
    gel3 = gout.tile([128, NE, TT], BF16, tag="gel3", bufs=1)

    xa_tiles = []   # per-tt xa_ext handles for tail chaining
    xc_tiles = {}   # tt -> xcb tile

    def emit_proj(tt):
        """x load + xa/gate projections + gelu + conv for tile tt."""
        xt = xpool.tile([128, NK, TT], BF16, tag="xt")
        xsrc = xT.ap().rearrange("(k p) t -> p k t", p=128)
        if tt == 0:
            nc.sync.dma_start(xt[:, 0], xsrc[:, 0, 0:TT])
            for k in range(NK):
                nc.sync.dma_start(win_sb[:, k], win_src[:, k])
                if k > 0:
                    nc.sync.dma_start(xt[:, k], xsrc[:, k, 0:TT])
        else:
            nc.sync.dma_start(xt[:], xsrc[:, :, tt * TT:(tt + 1) * TT])
        xa_ext = xa_pool.tile([128, NE, TT + KC - 1], BF16, tag="xa")
        xa_tiles.append(xa_ext)
        if tt == 0:
            nc.sync.dma_start(
                xa_ext[:, :, 0:KC - 1],
                xa_halo.ap().rearrange("(m p) n -> p m n", p=128))
        else:
            nc.vector.tensor_copy(xa_ext[:, :, 0:KC - 1],
                                  xa_tiles[tt - 1][:, :, TT:TT + KC - 1])
        xcb = xc_pool.tile([128, NE, TT], BF16, tag="xcb")
        xc_tiles[tt] = xcb
        for m in range(NE):
            pxa = ps1.tile([128, TT], F32, tag="ps")
            for k in range(NK):
                nc.tensor.matmul(pxa[:], win_sb[:, k, E + m * 128:E + (m + 1) * 128],
                                 xt[:, k], start=(k == 0), stop=(k == NK - 1))
            pg = ps1.tile([128, TT], F32, tag="ps")
            for k in range(NK):
                nc.tensor.matmul(pg[:], win_sb[:, k, m * 128:(m + 1) * 128],
                                 xt[:, k], start=(k == 0), stop=(k == NK - 1))
            nc.scalar.copy(xa_ext[:, m, KC - 1:TT + KC - 1], pxa[:])
            if tt == NTT - 1:
                nc.scalar.activation(gel3[:, m], pg[:], AF.Gelu, bias=c_zero[:])
            else:
                gel = gout.tile([128, TT], BF16, tag="gel", bufs=3)
                nc.scalar.activation(gel[:], pg[:], AF.Gelu, bias=c_zero[:])
                nc.sync.dma_start(g_dram[tt, :, m], gel[:])
            # causal depthwise conv, 4 taps, bf16 accumulate
            nc.vector.tensor_scalar(
                xcb[:, m], xa_ext[:, m, 0:TT], wc_sb[:, m, 0:1],
                b_conv_sb[:, m, 0:1], op0=OP.mult, op1=OP.add)
            for j in range(1, KC):
                nc.vector.scalar_tensor_tensor(
                    xcb[:, m], xa_ext[:, m, j:j + TT], wc_sb[:, m, j:j + 1],
                    xcb[:, m], op0=OP.mult, op1=OP.add)
        if tt == 0:
            for k in range(NK):
                nc.sync.dma_start(wg_sb[:, k], wg_src[:, k])

    def emit_gates(tt):
        """gates GEMM + activations + scans + spills for tile tt."""
        xcb = xc_tiles.pop(tt)
        pfs, pis = [], []
        for m in range(NE):
            pf = ps1.tile([128, TT], F32, tag="ps")
            for k in range(NK):
                nc.tensor.matmul(pf[:], wg_sb[:, k, m * 128:(m + 1) * 128],
                                 xcb[:, k], start=(k == 0), stop=(k == NK - 1))
            pfs.append(pf)
            pi = ps1.tile([128, TT], F32, tag="ps")
            for k in range(NK):
                nc.tensor.matmul(pi[:], wg_sb[:, k, E + m * 128:E + (m + 1) * 128],
                                 xcb[:, k], start=(k == 0), stop=(k == NK - 1))
            pis.append(pi)
        # sigmoid(x) = 0.5*tanh(x/2) + 0.5: Tanh and Exp share an ACT table,
        # so alpha = exp(c*sig(f)) = exp(c/2*tanh(f/2) + c/2) and
        # alpha^2 = exp(c*tanh(f/2) + c) cost zero table switches; only the
        # final Sqrt switches tables.
        sis, alphas, betas = {}, {}, {}
        for m in range(NE):
            thf = wpool.tile([128, TT], F32, tag="thf", bufs=2)
            nc.scalar.activation(thf[:], pfs[m][:], AF.Tanh,
                                 scale=0.5, bias=c_zero[:])
            thi = wpool.tile([128, TT], F32, tag="thi", bufs=2)
            nc.scalar.activation(thi[:], pis[m][:], AF.Tanh,
                                 scale=0.5, bias=c_zero[:])
            alpha = apool.tile([128, TT], F32, tag="alpha")
            nc.scalar.activation(alpha[:], thf[:], AF.Exp,
                                 scale=nch_sb[:, m, 0:1],
                                 bias=nch_sb[:, m, 0:1])
            alphas[m] = alpha
            asq = wpool.tile([128, TT], F32, tag="asq", bufs=8)
            nc.scalar.activation(asq[:], thf[:], AF.Exp,
                                 scale=neg_c_sb[:, m, 0:1],
                                 bias=neg_c_sb[:, m, 0:1])
            betas[m] = asq
            si = sig_pool.tile([128, TT], BF16, tag="si")
            nc.vector.tensor_scalar(si[:], thi[:], 0.5, 0.5,
                                    op0=OP.mult, op1=OP.add)
            sis[m] = si
        # sq_scale == -1.0, but computed from the last Exp output so every
        # Sqrt becomes ready only after the whole Exp phase — stops the
        # scheduler from interleaving Sqrt/Exp table reloads.
        sq_scale = wpool.tile([128, 1], F32, tag="sqs", bufs=2)
        nc.vector.tensor_scalar(sq_scale[:], betas[NE - 1][:, 0:1], 0.0, -1.0,
                                op0=OP.mult, op1=OP.add)
        for m in range(NE):
            nc.scalar.activation(betas[m][:], betas[m][:], AF.Sqrt,
                                 bias=c_sqb[:], scale=sq_scale[:, 0:1])
        for m in range(NE):
            bsi = wpool.tile([128, TT], F32, tag="bsi", bufs=2)
            nc.vector.tensor_mul(bsi[:], betas[m][:], sis[m][:])
            u = wpool.tile([128, TT], F32, tag="u", bufs=2)
            nc.vector.tensor_mul(u[:], bsi[:], xcb[:, m])
            nc.vector.tensor_tensor_scan(
                h_t[:, m], alphas[m][:], u[:],
                0.0 if tt == 0 else hl[:, m:m + 1],
                op0=OP.mult, op1=OP.add)
            nc.vector.tensor_copy(hl[:, m:m + 1], h_t[:, m, TT - 1:TT])
            nc.vector.tensor_tensor_scan(
                p_t[:, m], alphas[m][:], zeros[:],
                1.0 if tt == 0 else pl[:, m:m + 1],
                op0=OP.mult, op1=OP.add)
            nc.vector.tensor_copy(pl[:, m:m + 1], p_t[:, m, TT - 1:TT])
        if tt != NTT - 1:
            nc.sync.dma_start(h_dram[tt], h_t[:])
            nc.sync.dma_start(p_dram[tt], p_t[:])

    # pass 1, gates pipelined one tile behind the projections
    emit_proj(0)
    for tt in range(1, NTT):
        emit_proj(tt)
        emit_gates(tt - 1)
    emit_gates(NTT - 1)

    # ---- carry exchange (4KB pairwise AllReduce) ----
    contrib = consts.tile([128, NE], F32, tag="contrib")
    nc.vector.tensor_scalar(contrib[:], hl[:], mc_sb[:, 0:1], None, op0=OP.mult)
    nc.sync.dma_start(cc_in[:].rearrange("(j p) -> p j", p=128), contrib[:])
    if profile_mode:
        nc.sync.dma_start(cc_out[:], cc_in[:])
    else:
        nc.gpsimd.collective_compute(
            "AllReduce", OP.add,
            replica_groups=[[0, 1], [2, 3], [4, 5], [6, 7]],
            ins=[cc_in[:].opt()], outs=[cc_out[:].opt()])
    craw = consts.tile([128, NE], F32, tag="craw")
    nc.sync.dma_start(craw[:], cc_out[:].rearrange("(j p) -> p j", p=128))
    nc.vector.tensor_scalar(carry[:], craw[:], mu_sb[:, 0:1], None, op0=OP.mult)

    p1.close()

    # ---- pass 2: carry correction + y + output projection ----
    with ExitStack() as p2:
        wo_pool = p2.enter_context(tc.tile_pool(name="w_out", bufs=1, side="right"))
        wo_sb = wo_pool.tile([128, NE, DIM], BF16)
        wo_src = w_out.ap().rearrange("(m p) c -> p m c", p=128)
        for m in range(NE):
            nc.sync.dma_start(wo_sb[:, m], wo_src[:, m])
        gin_pool = p2.enter_context(tc.tile_pool(name="gin", bufs=2))
        hin_pool = p2.enter_context(tc.tile_pool(name="hin", bufs=2))
        pin_pool = p2.enter_context(tc.tile_pool(name="pin", bufs=2))
        ht_pool = p2.enter_context(tc.tile_pool(name="htp", bufs=3))
        y_pool = p2.enter_context(tc.tile_pool(name="y", bufs=9))
        osb_pool = p2.enter_context(tc.tile_pool(name="osb", bufs=3))
        ps2 = p2.enter_context(tc.tile_pool(name="ps2", bufs=8, space="PSUM"))
        for tt in [NTT - 1] + list(range(NTT - 1)):
            if tt == NTT - 1:
                gin, hin, pin = gel3, h_t, p_t
            else:
                gin = gin_pool.tile([128, NE, TT], BF16, tag="gin")
                nc.sync.dma_start(gin[:], g_dram[tt])
                hin = hin_pool.tile([128, NE, TT], BF16, tag="hin")
                nc.sync.dma_start(hin[:], h_dram[tt])
                pin = pin_pool.tile([128, NE, TT], BF16, tag="pin")
                nc.sync.dma_start(pin[:], p_dram[tt])
            ys = []
            for m in range(NE):
                htrue = ht_pool.tile([128, TT], BF16, tag="htrue")
                nc.vector.scalar_tensor_tensor(
                    htrue[:], pin[:, m], carry[:, m:m + 1], hin[:, m],
                    op0=OP.mult, op1=OP.add)
                y = y_pool.tile([128, TT], BF16, tag="y")
                if m % 2 == 0:
                    nc.gpsimd.tensor_mul(y[:], gin[:, m], htrue[:])
                else:
                    nc.vector.tensor_mul(y[:], gin[:, m], htrue[:])
                ys.append(y)
            for q in range(TT // 128):
                pos = [ps2.tile([128, 512], F32, tag="ps", name=f"po{n}") for n in range(2)]
                for m in range(NE):
                    for n in range(2):
                        nc.tensor.matmul(
                            pos[n][:], ys[m][:, q * 128:(q + 1) * 128],
                            wo_sb[:, m, n * 512:(n + 1) * 512],
                            start=(m == 0), stop=(m == NE - 1))
                osb = osb_pool.tile([128, DIM], BF16, tag="osb")
                for n in range(2):
                    nc.vector.tensor_copy(osb[:, n * 512:(n + 1) * 512], pos[n][:])
                nc.sync.dma_start(
                    out.ap()[tt * TT + q * 128:tt * TT + (q + 1) * 128, :],
                    osb[:])


_NC_CACHE = {}


def _get_nc():
    if "nc" not in _NC_CACHE:
        _NC_CACHE["nc"] = _build_kernel()
    return _NC_CACHE["nc"]


def _softplus(x):
    return np.logaddexp(0.0, x)


def kernel(x, w_in, w_conv, b_conv, w_gates, b_gates, forget_base, w_out,
           _want_trace=False):
    x = np.asarray(x, dtype=np.float32)
    w_in = np.asarray(w_in, dtype=np.float32)
    w_conv = np.asarray(w_conv, dtype=np.float32)
    b_conv = np.asarray(b_conv, dtype=np.float32)
    w_gates = np.asarray(w_gates, dtype=np.float32)
    forget_base = np.asarray(forget_base, dtype=np.float32)
    w_out = np.asarray(w_out, dtype=np.float32)

    nc = _get_nc()

    # [DIM, 2E]: cols 0:E gate-branch, E:2E x-branch
    w_in_cat = np.ascontiguousarray(
        np.concatenate([w_in[:E].T, w_in[E:].T], axis=1)).astype(
            ml_dtypes.bfloat16)
    w_gates_T = np.ascontiguousarray(w_gates.T).astype(ml_dtypes.bfloat16)
    w_out_T = np.ascontiguousarray(w_out.T).astype(ml_dtypes.bfloat16)
    wc_r = np.ascontiguousarray(w_conv.reshape(E, KC))
    neg_c = (-8.0 * _softplus(forget_base.astype(np.float64))).astype(
        np.float32)[:, None]

    common = {
        "w_in_cat": w_in_cat, "w_gates": w_gates_T, "w_out": w_out_T,
        "wc": wc_r, "b_conv": b_conv[:, None].copy(), "neg_c": neg_c,
        "nch": (0.5 * neg_c).copy(),
    }
    in_maps = []
    for k in range(N_CORES):
        b, half = k // 2, k % 2
        t0 = half * T_LOC
        xT_loc = np.ascontiguousarray(x[b, t0:t0 + T_LOC, :].T).astype(
            ml_dtypes.bfloat16)
        if half == 1:
            # xa for the 3 tokens before this chunk (for the causal conv)
            xa_halo = (x[b, t0 - (KC - 1):t0, :] @ w_in[E:].T).T
            xa_halo = np.ascontiguousarray(xa_halo).astype(ml_dtypes.bfloat16)
        else:
            xa_halo = np.zeros((E, KC - 1), dtype=ml_dtypes.bfloat16)
        mc = np.full((128, 1), 1.0 if half == 0 else 0.0, dtype=np.float32)
        mu = np.full((128, 1), 0.0 if half == 0 else 1.0, dtype=np.float32)
        in_maps.append({**common, "xT": xT_loc, "xa_halo": xa_halo,
                        "mask_c": mc, "mask_u": mu})

    res = run_bass_kernel_spmd(nc, in_maps, core_ids=list(range(N_CORES)),
                               trace=_want_trace)
    out_full = np.empty((B, T, DIM), dtype=np.float32)
    for k in range(N_CORES):
        b, half = k // 2, k % 2
        out_full[b, half * T_LOC:(half + 1) * T_LOC, :] = \
            res.results[k]["out"].astype(np.float32)
    if _want_trace:
        return out_full, res
    return out_full
